# revision 14
# baseline (speedup 1.0000x reference)
"""AUGRU (attention-modulated GRU) Trainium2 Bass kernel.

Problem: B=4096, T=200, D=H=128.  For each t:
  z = sigmoid([x,h] @ Wz + bz); r = sigmoid([x,h] @ Wr + br)
  h~ = tanh([x, r*h] @ Wh + bh); zp = a_t * z; h' = (1-zp)*h + zp*h~

Sharding: data-parallel over batch, B/8 = 512 rows per NeuronCore.
Per-core layout: state hT [128(h), 512(b)] fp16 in SBUF, two half-batch
chains pipelined ~1/4-step apart.

Key structure (chosen from trace measurements):
- Per-chain fused z|r PSUM bank [128, 512] = [z(256) | r(256)]: opened
  by a K=2 bias matmul (lhsT=[bz;br] stacked, rhs=0/1 indicator rows,
  so bias lands in PSUM), then x-part, Wh@h(t-1) (off-path), and
  Wh@m(t-1) (on-path delta) accumulate.  This allows ONE fused FD=512
  sigmoid per chain per step (ACT ops 6 -> 4/step; the ScalarE's
  172-222cyc fixed cost per instruction made ACT the throughput wall),
  and makes z available as early as r, shortening the z->q->m chain.
- Delta-form state feed ("hoist"): bank(t+1) = x + bias + Wh@h(t-1)
  + Wh@m(t), so the h' = h+m add is off the critical path.
- All elementwise ops on DVE (~203ns each at 2x mode).  GPSIMD
  tensor ops measured ~700-800ns exec + ~900ns completion-join/sem
  latency - unusable inside a <3us step.
- Keep-warm dummies must be full-K matmuls: the HAM clock monitor
  tracks PE array activity; K=1 outer products do NOT hold 2.4GHz.
"""

import numpy as np

B, T, D, H = 4096, 200, 128, 128
NCORES = 8
BS = B // NCORES            # 512 batch rows per core
C = 10                      # timestep chunk for attention staging

F16 = np.float16

_compiled = None


def _build(t_steps=T, chunk=C, ndum=3, ndw=256):
    import concourse.bass as bass
    import concourse.bacc as bacc
    import concourse.mybir as mybir
    from concourse.tile import TileContext
    from concourse.bass_types import AP

    fp32 = mybir.dt.float32
    fp16 = mybir.dt.float16
    Sigmoid = mybir.ActivationFunctionType.Sigmoid
    Tanh = mybir.ActivationFunctionType.Tanh

    assert t_steps % chunk == 0
    nchunks = t_steps // chunk

    nco = bacc.Bacc(
        "TRN2", target_bir_lowering=False, debug=False, num_devices=NCORES
    )
    xt_d = nco.dram_tensor("xt", [t_steps, D, BS], fp16, kind="ExternalInput")
    a_d = nco.dram_tensor("abf", [t_steps, BS], fp16, kind="ExternalInput")
    h0_d = nco.dram_tensor("h0t", [H, BS], fp16, kind="ExternalInput")
    wx_d = nco.dram_tensor("wx", [D, 3 * H], fp16, kind="ExternalInput")
    wh_d = nco.dram_tensor("wh", [H, 3 * H], fp16, kind="ExternalInput")
    b2_d = nco.dram_tensor("b2", [2, H], fp16, kind="ExternalInput")
    ind_d = nco.dram_tensor("ind", [2, BS], fp16, kind="ExternalInput")
    b_d = nco.dram_tensor("bcol", [H, 4], fp32, kind="ExternalInput")
    out_d = nco.dram_tensor("out", [H, BS], fp16, kind="ExternalOutput")

    GSL = {"z": slice(0, 128), "r": slice(128, 256), "p": slice(256, 384)}
    NCH = 2
    CW = BS // NCH

    with TileContext(nco) as tc:
        with (
            tc.tile_pool(name="const", bufs=1) as constp,
            tc.tile_pool(name="xT", bufs=6) as xTp,
            tc.tile_pool(name="ab", bufs=2) as abp,
            tc.tile_pool(name="state", bufs=4) as statep,
            tc.tile_pool(name="tmp", bufs=3) as tmpp,
            tc.tile_pool(name="zr", bufs=3) as zrp,
            tc.tile_pool(name="psg", bufs=2, space="PSUM") as psg,
            tc.tile_pool(name="psd", bufs=2, space="PSUM") as psd,
        ):
            mm = nco.tensor.matmul

            # ---- constants ----
            wx_sb = constp.tile([128, 3 * H], fp16, tag="wx")
            nco.sync.dma_start(out=wx_sb[:], in_=wx_d.ap())
            wh_sb = constp.tile([128, 3 * H], fp16, tag="wh")
            nco.sync.dma_start(out=wh_sb[:], in_=wh_d.ap())
            b2_sb = constp.tile([2, H], fp16, tag="b2")
            nco.sync.dma_start(out=b2_sb[:], in_=b2_d.ap())
            ind_sb = constp.tile([2, BS], fp16, tag="ind")
            nco.sync.dma_start(out=ind_sb[:], in_=ind_d.ap())
            b_sb = constp.tile([128, 4], fp32, tag="bcol")
            nco.sync.dma_start(out=b_sb[:], in_=b_d.ap())

            hTs = []
            for c in range(NCH):
                hT = statep.tile([128, CW], fp16, tag=f"h{c}")
                nco.sync.dma_start(
                    out=hT[:], in_=h0_d.ap()[:, c * CW : (c + 1) * CW]
                )
                hTs.append(hT[:])

            # Chunked attention broadcast: one zero-stride DMA replicates
            # a[t0:t0+chunk, :] across all 128 partitions.
            def abload(t0):
                ab_ch = abp.tile([128, chunk, BS], fp16, tag="ab")
                asrc = a_d.ap()[t0 : t0 + chunk, :]
                asrc = AP(asrc.tensor, asrc.offset, [[0, 128]] + list(asrc.ap))
                nco.sync.dma_start(out=ab_ch[:], in_=asrc)
                return ab_ch

            ab_chunks = [None] * nchunks
            ab_chunks[0] = abload(0)
            if nchunks > 1:
                ab_chunks[1] = abload(chunk)

            banks = {}  # (t, key) -> psum tile; keys "zr0","zr1","p"
            st = [dict(hT=hTs[c]) for c in range(NCH)]

            def cwsl(c):
                return slice(c * CW, (c + 1) * CW)

            # ---- emitters ----
            def xload(t):
                xT_t = xTp.tile([128, BS], fp16, tag="xT")
                nco.sync.dma_start(out=xT_t[:], in_=xt_d.ap()[t])
                return xT_t[:]

            def pbias(c, t):
                # K=2 bias matmul opens chain-c's z|r bank with the biases:
                # out[h, n] = bz[h]*ind0[n] + br[h]*ind1[n].
                bank = psg.tile([128, BS], fp32, tag=f"zr{c}")
                banks[(t, f"zr{c}")] = bank
                mm(bank[:], b2_sb[:], ind_sb[:], start=True, stop=False)

            def p1xzr(c, t, xT_t):
                bank = banks[(t, f"zr{c}")]
                cw = cwsl(c)
                mm(bank[:, 0:CW], wx_sb[:, GSL["z"]], xT_t[:, cw],
                   start=False, stop=False, skip_group_check=True)
                mm(bank[:, CW:], wx_sb[:, GSL["r"]], xT_t[:, cw],
                   start=False, stop=False, skip_group_check=True)

            def p1xp(t, xT_t):
                bank = psg.tile([128, BS], fp32, tag="p")
                banks[(t, "p")] = bank
                mm(bank[:], wx_sb[:, GSL["p"]], xT_t, start=True, stop=False)

            def p1hold(c, t, stop=False):
                # Off-path state feed: Wh@h(t-1) into step t's zr bank.
                s_ = st[c]
                bank = banks[(t, f"zr{c}")]
                mm(bank[:, CW:], wh_sb[:, GSL["r"]], s_["hT"],
                   start=False, stop=False, skip_group_check=True)
                mm(bank[:, 0:CW], wh_sb[:, GSL["z"]], s_["hT"],
                   start=False, stop=stop, skip_group_check=True)

            def mmzr(c, t):
                # On-path delta feed: Wh@m(t-1) closes step t's zr bank.
                s_ = st[c]
                bank = banks[(t, f"zr{c}")]
                mm(bank[:, CW:], wh_sb[:, GSL["r"]], s_["m"],
                   start=False, stop=False, skip_group_check=True)
                mm(bank[:, 0:CW], wh_sb[:, GSL["z"]], s_["m"],
                   start=False, stop=True, skip_group_check=True)

            def sigzr(c, t):
                # ONE fused sigmoid over [z|r] (biases already in PSUM).
                s_ = st[c]
                s_["ab"] = ab_chunks[t // chunk][:, t % chunk, cwsl(c)]
                zr_bf = zrp.tile([128, 2 * CW], fp16, tag=f"zr{c}")
                nco.scalar.activation(
                    zr_bf[:], banks[(t, f"zr{c}")][:], Sigmoid
                )
                s_["zr_bf"] = zr_bf

            def rh(c):
                s_ = st[c]
                rh_ = tmpp.tile([128, CW], fp16, tag=f"rh{c}")
                nco.vector.tensor_mul(rh_[:], s_["zr_bf"][:, CW:], s_["hT"])
                s_["rh"] = rh_[:]

            def qq(c):
                s_ = st[c]
                q_ = tmpp.tile([128, CW], fp16, tag=f"q{c}")
                nco.vector.tensor_mul(q_[:], s_["ab"], s_["zr_bf"][:, 0:CW])
                s_["q"] = q_[:]

            def mmp(c, t):
                s_ = st[c]
                mm(banks[(t, "p")][:, cwsl(c)], wh_sb[:, GSL["p"]], s_["rh"],
                   start=False, stop=(c == NCH - 1), skip_group_check=True)

            def tanh(c, t):
                s_ = st[c]
                ht_ = tmpp.tile([128, CW], fp16, tag=f"ht{c}")
                nco.scalar.activation(
                    ht_[:], banks[(t, "p")][:, cwsl(c)], Tanh, bias=b_sb[:, 2:3]
                )
                s_["ht"] = ht_[:]

            def dsub(c):
                s_ = st[c]
                dd = tmpp.tile([128, CW], fp16, tag=f"d{c}")
                nco.vector.tensor_sub(dd[:], s_["ht"], s_["hT"])
                s_["d"] = dd[:]

            def mmul(c):
                s_ = st[c]
                mt = tmpp.tile([128, CW], fp16, tag=f"m{c}")
                nco.vector.tensor_mul(mt[:], s_["q"], s_["d"])
                s_["m"] = mt[:]

            def hupd(c):
                s_ = st[c]
                hT_new = statep.tile([128, CW], fp16, tag=f"h{c}")
                nco.vector.tensor_add(hT_new[:], s_["hT"], s_["m"])
                hTs[c] = hT_new[:]
                s_["hT"] = hTs[c]

            # Keep-warm filler (full-K matmuls; see module docstring).
            def dummies(n):
                for _ in range(n):
                    db = psd.tile([128, ndw], fp32, tag="dum")
                    mm(db[:], wx_sb[:, 0:128], wh_sb[:, 0:ndw],
                       start=True, stop=True)

            # ---- preamble: step 0 banks ----
            xts = {0: xload(0)}
            if t_steps > 1:
                xts[1] = xload(1)
            for c in range(NCH):
                pbias(c, 0)
                p1xzr(c, 0, xts[0])
            p1xp(0, xts[0])
            p1hold(0, 0, stop=True)
            p1hold(1, 0, stop=True)

            # ---- main loop ----
            # Both chains advance within each iteration, ~1/4 step apart;
            # every op emitted in expected-ready-time order per engine.
            for t in range(t_steps):
                nxt = t // chunk + 1
                if t % chunk == 0 and nxt < nchunks and ab_chunks[nxt] is None:
                    ab_chunks[nxt] = abload(t + chunk)
                if t + 2 < t_steps:
                    xts[t + 2] = xload(t + 2)

                last = t + 1 >= t_steps

                sigzr(0, t)                    # ACT: fused z|r, chain A
                if t > 0:
                    mmzr(1, t)                 # PE crit: close B's zr bank
                rh(0)                          # DVE
                qq(0)                          # DVE
                sigzr(1, t)                    # ACT: fused z|r, chain B
                mmp(0, t)                      # PE crit
                dummies(1)
                tanh(0, t)                     # ACT
                rh(1)                          # DVE
                qq(1)                          # DVE
                mmp(1, t)                      # PE crit
                if not last:
                    pbias(0, t + 1)            # PE fillers for step t+1
                    pbias(1, t + 1)
                    p1xzr(0, t + 1, xts[t + 1])
                    p1xzr(1, t + 1, xts[t + 1])
                    p1xp(t + 1, xts[t + 1])
                    p1hold(0, t + 1)           # uses h_A(t-1): before hupd(0)
                tanh(1, t)                     # ACT
                dsub(0)                        # DVE
                mmul(0)                        # DVE
                if not last:
                    mmzr(0, t + 1)             # PE crit: Wh@m_A into t+1
                hupd(0)                        # DVE
                if not last:
                    p1hold(1, t + 1)           # uses h_B(t-1): before hupd(1)
                dsub(1)                        # DVE
                mmul(1)                        # DVE
                hupd(1)                        # DVE
                dummies(ndum - 1)

                for g in ("zr0", "zr1", "p"):
                    banks.pop((t - 1, g), None)
                xts.pop(t - 1, None)

            # ---- store final state transposed [H, BS] fp16; host flips ----
            for c in range(NCH):
                nco.gpsimd.dma_start(
                    out=out_d.ap()[:, c * CW : (c + 1) * CW], in_=hTs[c]
                )

    nco.compile()
    return nco


def _in_maps(inputs, t_steps=T):
    x = np.asarray(inputs["inputs"], np.float32)
    a = np.asarray(inputs["attention_scores"], np.float32)
    h0 = np.asarray(inputs["h0"], np.float32)
    Wz = np.asarray(inputs["Wz"], np.float32)
    Wr = np.asarray(inputs["Wr"], np.float32)
    Wh = np.asarray(inputs["Wh"], np.float32)
    wx = np.concatenate([Wz[:D], Wr[:D], Wh[:D]], axis=1).astype(F16)
    wh = np.concatenate([Wz[D:], Wr[D:], Wh[D:]], axis=1).astype(F16)
    b2 = np.stack(
        [np.asarray(inputs["bz"], np.float32), np.asarray(inputs["br"], np.float32)]
    ).astype(F16)
    ind = np.zeros((2, BS), np.float16)
    ind[0, : BS // 2] = 1.0
    ind[1, BS // 2 :] = 1.0
    bcol = np.zeros((H, 4), np.float32)
    for i, k in enumerate(("bz", "br", "bh")):
        bcol[:, i] = np.asarray(inputs[k], np.float32)
    maps = []
    for c in range(NCORES):
        sl = slice(c * BS, (c + 1) * BS)
        maps.append(
            {
                "xt": np.ascontiguousarray(
                    x[sl, :t_steps].transpose(1, 2, 0)
                ).astype(F16),
                "abf": np.ascontiguousarray(a[sl, :t_steps].T).astype(F16),
                "h0t": np.ascontiguousarray(h0[sl].T).astype(F16),
                "wx": wx,
                "wh": wh,
                "b2": b2,
                "ind": ind,
                "bcol": bcol,
            }
        )
    return maps


def kernel(**inputs):
    global _compiled
    from concourse.bass_utils import run_bass_kernel_spmd

    if _compiled is None:
        _compiled = _build()
    res = run_bass_kernel_spmd(_compiled, _in_maps(inputs), core_ids=list(range(NCORES)))
    return np.ascontiguousarray(
        np.concatenate(
            [np.asarray(r["out"]).astype(np.float32).T for r in res.results], axis=0
        )
    )


# revision 15
# speedup vs baseline: 1.2597x; 1.2597x over previous
"""AUGRU (attention-modulated GRU) Trainium2 Bass kernel.

Problem: B=4096, T=200, D=H=128.  For each t:
  z = sigmoid([x,h] @ Wz + bz); r = sigmoid([x,h] @ Wr + br)
  h~ = tanh([x, r*h] @ Wh + bh); zp = a_t * z; h' = (1-zp)*h + zp*h~

Sharding: data-parallel over batch, B/8 = 512 rows per NeuronCore.

Host-side prep (inside kernel(), before dispatch): x is transposed to
[T, D, B_shard] fp16 (the matmul moving operand), attention scores
[T, B_shard] fp16, h0 [H, B_shard] fp16, weights split into x-part /
h-part fp16.

Per-core device layout: state hT [128(h), 512(b)] fp16 in SBUF, two
half-batch chains staggered half a step.  PSUM holds one bank per gate
per step ([128,512] fp32, double buffered = 6 banks): a single N=512
x-part matmul (start=True) opens each bank for BOTH chains; per-chain
accumulations use skip_group_check (one open group per bank at a time).

Delta-form state feed (always-on "hoist"): the z|r banks for step t+1
receive Wh@h(t) as Wh@h(t-1) [emitted early, off the critical path]
plus Wh@m(t) [on-path, where m = zp*(h~-h)].  This takes the final
h' = h + m add (and one sem hop) off the recurrence's critical path.

Emission order is tuned to the Tile list-scheduler's priorities: each
engine executes its stream in emission order, so every op is emitted in
expected-ready-time order (DVE: the other chain's d before this chain's
rh, etc.), critical matmuls are emitted before PE filler, and dummies
are small (N<=256) so they block a waiting critical matmul as little
as possible while still holding the PE's 2.4GHz p-state.
"""

import numpy as np

B, T, D, H = 4096, 200, 128, 128
NCORES = 8
BS = B // NCORES            # 512 batch rows per core
C = 10                      # timestep chunk for attention staging

F16 = np.float16

_compiled = None


def _build(t_steps=T, chunk=C, ndum=6, ndw=256):
    import concourse.bass as bass
    import concourse.bacc as bacc
    import concourse.mybir as mybir
    from concourse.tile import TileContext
    from concourse.bass_types import AP

    fp32 = mybir.dt.float32
    fp16 = mybir.dt.float16
    Sigmoid = mybir.ActivationFunctionType.Sigmoid
    Tanh = mybir.ActivationFunctionType.Tanh

    assert t_steps % chunk == 0
    nchunks = t_steps // chunk

    nco = bacc.Bacc(
        "TRN2", target_bir_lowering=False, debug=False, num_devices=NCORES
    )
    xt_d = nco.dram_tensor("xt", [t_steps, D, BS], fp16, kind="ExternalInput")
    a_d = nco.dram_tensor("abf", [t_steps, BS], fp16, kind="ExternalInput")
    h0_d = nco.dram_tensor("h0t", [H, BS], fp16, kind="ExternalInput")
    wx_d = nco.dram_tensor("wx", [D, 3 * H], fp16, kind="ExternalInput")
    wh_d = nco.dram_tensor("wh", [H, 3 * H], fp16, kind="ExternalInput")
    b_d = nco.dram_tensor("bcol", [H, 4], fp32, kind="ExternalInput")
    out_d = nco.dram_tensor("out", [H, BS], fp16, kind="ExternalOutput")

    GSL = {"z": slice(0, 128), "r": slice(128, 256), "p": slice(256, 384)}
    NCH = 2
    CW = BS // NCH

    with TileContext(nco) as tc:
        with (
            tc.tile_pool(name="const", bufs=1) as constp,
            tc.tile_pool(name="xT", bufs=6) as xTp,
            tc.tile_pool(name="ab", bufs=2) as abp,
            tc.tile_pool(name="state", bufs=4) as statep,
            tc.tile_pool(name="tmp", bufs=3) as tmpp,
            tc.tile_pool(name="zr", bufs=3) as zrp,
            tc.tile_pool(name="psg", bufs=2, space="PSUM") as psg,
            tc.tile_pool(name="psd", bufs=2, space="PSUM") as psd,
        ):
            mm = nco.tensor.matmul

            # ---- constants ----
            wx_sb = constp.tile([128, 3 * H], fp16, tag="wx")
            nco.sync.dma_start(out=wx_sb[:], in_=wx_d.ap())
            wh_sb = constp.tile([128, 3 * H], fp16, tag="wh")
            nco.sync.dma_start(out=wh_sb[:], in_=wh_d.ap())
            b_sb = constp.tile([128, 4], fp32, tag="bcol")
            nco.sync.dma_start(out=b_sb[:], in_=b_d.ap())

            hTs = []
            for c in range(NCH):
                hT = statep.tile([128, CW], fp16, tag=f"h{c}")
                nco.sync.dma_start(
                    out=hT[:], in_=h0_d.ap()[:, c * CW : (c + 1) * CW]
                )
                hTs.append(hT[:])

            # Chunked attention broadcast: one zero-stride DMA replicates
            # a[t0:t0+chunk, :] across all 128 partitions.
            def abload(t0):
                ab_ch = abp.tile([128, chunk, BS], fp16, tag="ab")
                asrc = a_d.ap()[t0 : t0 + chunk, :]
                asrc = AP(asrc.tensor, asrc.offset, [[0, 128]] + list(asrc.ap))
                nco.sync.dma_start(out=ab_ch[:], in_=asrc)
                return ab_ch

            ab_chunks = [None] * nchunks
            ab_chunks[0] = abload(0)
            if nchunks > 1:
                ab_chunks[1] = abload(chunk)

            banks = {}  # (t, gate) -> psum tile [128, BS]
            st = [dict(hT=hTs[c]) for c in range(NCH)]

            def cwsl(c):
                return slice(c * CW, (c + 1) * CW)

            # ---- emitters (one engine-op each, fine-grained) ----
            def xload(t):
                xT_t = xTp.tile([128, BS], fp16, tag="xT")
                nco.sync.dma_start(out=xT_t[:], in_=xt_d.ap()[t])
                return xT_t[:]

            def p1x(g, t, xT_t):
                # x-part matmul: one N=512 start=True opens the bank for
                # both chains (single open group per bank).
                bank = psg.tile([128, BS], fp32, tag=f"ps{g}")
                banks[(t, g)] = bank
                mm(bank[:], wx_sb[:, GSL[g]], xT_t, start=True, stop=False)

            def p1hold(c, t, stop=False):
                # Off-path state feed: Wh@h(t-1) into step t's z|r banks.
                s_ = st[c]
                cw = cwsl(c)
                mm(banks[(t, "r")][:, cw], wh_sb[:, GSL["r"]], s_["hT"],
                   start=False, stop=False, skip_group_check=True)
                mm(banks[(t, "z")][:, cw], wh_sb[:, GSL["z"]], s_["hT"],
                   start=False, stop=stop, skip_group_check=True)

            def mmzr(c, t, stop):
                # On-path delta feed: Wh@m(t-1) closes step t's z|r banks.
                s_ = st[c]
                cw = cwsl(c)
                mm(banks[(t, "r")][:, cw], wh_sb[:, GSL["r"]], s_["m"],
                   start=False, stop=False, skip_group_check=True)
                mm(banks[(t, "z")][:, cw], wh_sb[:, GSL["z"]], s_["m"],
                   start=False, stop=stop, skip_group_check=True)

            def sigr(c, t):
                s_ = st[c]
                cw = cwsl(c)
                s_["ab"] = ab_chunks[t // chunk][:, t % chunk, cw]
                zr_bf = zrp.tile([128, 2 * CW], fp16, tag=f"zr{c}")
                nco.scalar.activation(
                    zr_bf[:, CW:], banks[(t, "r")][:, cw], Sigmoid,
                    bias=b_sb[:, 1:2]
                )
                s_["zr_bf"] = zr_bf

            def sigz(c, t):
                s_ = st[c]
                cw = cwsl(c)
                nco.scalar.activation(
                    s_["zr_bf"][:, 0:CW], banks[(t, "z")][:, cw], Sigmoid,
                    bias=b_sb[:, 0:1]
                )

            def rh(c):
                s_ = st[c]
                rh_ = tmpp.tile([128, CW], fp16, tag=f"rh{c}")
                nco.vector.tensor_mul(rh_[:], s_["zr_bf"][:, CW:], s_["hT"])
                s_["rh"] = rh_[:]

            def qq(c):
                # q = a*z on DVE: GPSIMD's ~700-800ns exec + ~900ns
                # completion-join/sem latency put the z->q->m chain on the
                # step's critical cycle (measured).  DVE is ~250ns + ~100.
                s_ = st[c]
                q_ = tmpp.tile([128, CW], fp16, tag=f"q{c}")
                nco.vector.tensor_mul(q_[:], s_["ab"], s_["zr_bf"][:, 0:CW])
                s_["q"] = q_[:]

            def mmp(c, t):
                s_ = st[c]
                cw = cwsl(c)
                mm(banks[(t, "p")][:, cw], wh_sb[:, GSL["p"]], s_["rh"],
                   start=False, stop=(c == NCH - 1), skip_group_check=True)

            def tanh(c, t):
                s_ = st[c]
                cw = cwsl(c)
                ht_ = tmpp.tile([128, CW], fp16, tag=f"ht{c}")
                nco.scalar.activation(
                    ht_[:], banks[(t, "p")][:, cw], Tanh, bias=b_sb[:, 2:3]
                )
                s_["ht"] = ht_[:]

            def dsub(c):
                s_ = st[c]
                dd = tmpp.tile([128, CW], fp16, tag=f"d{c}")
                nco.vector.tensor_sub(dd[:], s_["ht"], s_["hT"])
                s_["d"] = dd[:]

            def mmul(c):
                s_ = st[c]
                mt = tmpp.tile([128, CW], fp16, tag=f"m{c}")
                nco.vector.tensor_mul(mt[:], s_["q"], s_["d"])
                s_["m"] = mt[:]

            def hupd(c):
                # h' = h + m on DVE, placed late in the stream where it
                # doubles as a poll-absorber before the next d-sub.
                s_ = st[c]
                hT_new = statep.tile([128, CW], fp16, tag=f"h{c}")
                nco.vector.tensor_add(hT_new[:], s_["hT"], s_["m"])
                hTs[c] = hT_new[:]
                s_["hT"] = hTs[c]

            # Keep-warm filler: the PE drops from 2.4GHz to 1.2GHz whenever
            # its pipeline drains, and a critical matmul arriving at an
            # EMPTY PE queue pays its LDWEIGHTS exposed (~95-190ns).
            # Dummies must be FULL-K (128-row) matmuls: the HAM clock
            # monitor tracks array activity, and K=1 outer-product dummies
            # (1 active row) fail to hold the 2.4GHz p-state (measured:
            # whole kernel drops to 1.2GHz).  Placed just before late-ready
            # critical matmuls they also keep the queue non-empty so the
            # critical LDW preloads during the dummy's array time.
            def dummies(n):
                for _ in range(n):
                    db = psd.tile([128, ndw], fp32, tag="dum")
                    mm(db[:], wx_sb[:, 0:128], wh_sb[:, 0:ndw],
                       start=True, stop=True)

            # ---- preamble: step 0 banks ----
            xts = {0: xload(0)}
            if t_steps > 1:
                xts[1] = xload(1)
            for g in ("r", "z", "p"):
                p1x(g, 0, xts[0])
            p1hold(0, 0)
            p1hold(1, 0, stop=True)

            # ---- main loop ----
            # Chain B (c=1) runs half a step behind chain A (c=0).  All
            # emissions within iter t are ordered by expected ready time.
            for t in range(t_steps):
                nxt = t // chunk + 1
                if t % chunk == 0 and nxt < nchunks and ab_chunks[nxt] is None:
                    ab_chunks[nxt] = abload(t + chunk)
                if t + 2 < t_steps:
                    xts[t + 2] = xload(t + 2)

                last = t + 1 >= t_steps

                sigr(0, t)                     # ACT: r_A(t)
                sigz(0, t)                     # ACT: z_A(t) (feeds q-path)
                if t > 0:
                    dsub(1)                    # DVE: d_B(t-1)
                    mmul(1)                    # DVE: m_B(t-1)
                    hupd(1)                    # GPSIMD: h'_B(t-1)
                rh(0)                          # DVE: rh_A(t)
                qq(0)                          # DVE: q_A(t)
                if t > 0:
                    mmzr(1, t, stop=True)      # PE crit: close z|r bank t
                mmp(0, t)                      # PE crit: p bank, A half
                dummies(1)
                sigr(1, t)                     # ACT: r_B(t)
                tanh(0, t)                     # ACT: h~_A(t)
                rh(1)                          # DVE: rh_B(t)
                sigz(1, t)                     # ACT: z_B(t)
                dsub(0)                        # DVE: d_A(t)
                qq(1)                          # DVE: q_B(t)
                mmul(0)                        # DVE: m_A(t)
                if not last:
                    p1x("z", t + 1, xts[t + 1])   # PE filler block: runs
                    p1x("r", t + 1, xts[t + 1])   # during B's ACT stretch
                    p1x("p", t + 1, xts[t + 1])
                    p1hold(0, t + 1)           # PE filler: Wh@h_A(t-1)
                hupd(0)                        # GPSIMD: h'_A(t)
                mmp(1, t)                      # PE crit: p bank, B half
                if not last:
                    mmzr(0, t + 1, stop=False) # PE crit: Wh@m_A into t+1
                    p1hold(1, t + 1)           # PE filler: Wh@h_B(t-1)
                tanh(1, t)                     # ACT: h~_B(t)
                dummies(ndum - 1)

                for g in ("z", "r", "p"):
                    banks.pop((t - 1, g), None)
                xts.pop(t - 1, None)

            # ---- drain chain B's final step ----
            dsub(1)
            mmul(1)
            hupd(1)

            # ---- store final state transposed [H, BS] fp16; host flips ----
            for c in range(NCH):
                nco.gpsimd.dma_start(
                    out=out_d.ap()[:, c * CW : (c + 1) * CW], in_=hTs[c]
                )

    nco.compile()
    return nco


def _in_maps(inputs, t_steps=T):
    x = np.asarray(inputs["inputs"], np.float32)
    a = np.asarray(inputs["attention_scores"], np.float32)
    h0 = np.asarray(inputs["h0"], np.float32)
    Wz = np.asarray(inputs["Wz"], np.float32)
    Wr = np.asarray(inputs["Wr"], np.float32)
    Wh = np.asarray(inputs["Wh"], np.float32)
    wx = np.concatenate([Wz[:D], Wr[:D], Wh[:D]], axis=1).astype(F16)
    wh = np.concatenate([Wz[D:], Wr[D:], Wh[D:]], axis=1).astype(F16)
    bcol = np.zeros((H, 4), np.float32)
    for i, k in enumerate(("bz", "br", "bh")):
        bcol[:, i] = np.asarray(inputs[k], np.float32)
    maps = []
    for c in range(NCORES):
        sl = slice(c * BS, (c + 1) * BS)
        maps.append(
            {
                "xt": np.ascontiguousarray(
                    x[sl, :t_steps].transpose(1, 2, 0)
                ).astype(F16),
                "abf": np.ascontiguousarray(a[sl, :t_steps].T).astype(F16),
                "h0t": np.ascontiguousarray(h0[sl].T).astype(F16),
                "wx": wx,
                "wh": wh,
                "bcol": bcol,
            }
        )
    return maps


def kernel(**inputs):
    global _compiled
    from concourse.bass_utils import run_bass_kernel_spmd

    if _compiled is None:
        _compiled = _build()
    res = run_bass_kernel_spmd(_compiled, _in_maps(inputs), core_ids=list(range(NCORES)))
    return np.ascontiguousarray(
        np.concatenate(
            [np.asarray(r["out"]).astype(np.float32).T for r in res.results], axis=0
        )
    )
